# revision 1
# baseline (speedup 1.0000x reference)
"""Trainium2 Bass kernel for per-sample modulated+demodulated 3D conv.

Problem: x[B=8, CIN=128, 32,32,32], y[8,128], weight[128,128,3,3,3] (shared).
  w_b = weight * (1 + y[b,i]);  w_b *= rsqrt(sum_{i,k} w_b^2 + eps)  (per out-ch)
  out[b] = conv3d(x[b], w_b, same padding)

Sharding: data-parallel over batch, one sample per NeuronCore (8 cores).
Per core: 27 shift-matmuls ([CIN=128] x [COUT=128] stationary against an
H/W-zero-padded x volume) accumulate each PSUM tile; demodulation is folded
into the PSUM->SBUF evacuation as a per-partition scale. Matmul operands are
fp16 (full PE rate, 10-bit mantissa; fp32 PSUM accumulation).
"""

import sys

import numpy as np

try:
    import concourse.bass as bass
except ImportError:  # fresh grading dir: fall back to the repo checkout
    sys.path.insert(0, "/opt/trn_rl_repo")
    import concourse.bass as bass

import concourse.tile as tile
from concourse import bacc, mybir
from concourse.masks import make_identity

B, CIN, COUT, K = 8, 128, 128, 3
D = H = W = 32
T = K * K * K  # 27
HP = H + 2  # padded rows
WP = W + 2  # padded cols
EPS = 1e-8
N_CORES = 8

FP32 = mybir.dt.float32
MM_DT = mybir.dt.float16  # matmul operand dtype (fp32 PSUM accumulate)

_CACHE = {}


def _build_program():
    nc = bacc.Bacc()
    xv = nc.dram_tensor("x", [CIN, D, H, W], FP32, kind="ExternalInput")
    yv = nc.dram_tensor("y", [CIN, 1], FP32, kind="ExternalInput")
    wv = nc.dram_tensor("w", [COUT, CIN, T], FP32, kind="ExternalInput")
    ov = nc.dram_tensor("out", [COUT, D, H * W], FP32, kind="ExternalOutput")

    PLANES_PER_CHUNK = 4
    N_CHUNKS = D // PLANES_PER_CHUNK

    with tile.TileContext(nc) as tc:
        with (
            tc.tile_pool(name="const", bufs=1) as const,
            tc.tile_pool(name="xnat", bufs=2) as xnat,
            tc.tile_pool(name="evac", bufs=4) as evac,
            tc.tile_pool(name="psum", bufs=4, space="PSUM") as psum,
            tc.tile_pool(name="psw", bufs=4, space="PSUM") as psw,
        ):
            # ---------------- x staging (issue first: overlaps weight prep) ----
            ROT = 8
            pads = []
            for r in range(ROT):
                pt = const.tile([CIN, HP, WP], MM_DT, tag=f"pad{r}", name=f"pad{r}")
                pads.append(pt)

            pad_tiles = [None] * D
            nat_tiles = [None] * N_CHUNKS

            def stage_chunk(c):
                nat = xnat.tile(
                    [CIN, PLANES_PER_CHUNK, H, W], FP32, tag="xnat", name="nat"
                )
                p0 = c * PLANES_PER_CHUNK
                nc.sync.dma_start(out=nat, in_=xv[:, p0 : p0 + PLANES_PER_CHUNK, :, :])
                nat_tiles[c] = nat

            def stage_plane(p):
                c, pi = divmod(p, PLANES_PER_CHUNK)
                if nat_tiles[c] is None:
                    stage_chunk(c)
                pad = pads[p % ROT]
                nc.vector.tensor_copy(
                    pad[:, 1 : HP - 1, 1 : WP - 1], nat_tiles[c][:, pi, :, :]
                )
                pad_tiles[p] = pad

            stage_chunk(0)  # x DMA in flight while weights prep

            # zero pad borders once (interior repack never touches them)
            ztile = const.tile([CIN, HP, WP], FP32, tag="ztile")
            nc.vector.memset(ztile, 0.0)
            for pt in pads:
                nc.vector.tensor_copy(pt, ztile)

            # ---------------- weight prep ----------------
            wa = const.tile([COUT, CIN, T], FP32, tag="wa")
            nc.sync.dma_start(out=wa, in_=wv[:, :, :])
            ym = const.tile([CIN, 1], FP32, tag="ym")
            nc.sync.dma_start(out=ym, in_=yv[:, :])
            yrow = const.tile([1, CIN], FP32, tag="yrow")
            nc.sync.dma_start(out=yrow, in_=yv[:, 0])

            # (1 + y[i]) as a per-partition column (for modulation)
            ymp1 = const.tile([CIN, 1], FP32, tag="ymp1")
            nc.vector.tensor_scalar_add(ymp1, ym, 1.0)

            ident = const.tile([128, 128], FP32, tag="ident")
            make_identity(nc, ident)

            # transposed modulated weights: wT3[i, t, o] = w[o,i,t]*(1+y[i])
            wT3 = const.tile([CIN, T, COUT], MM_DT, tag="wT3")
            for t in range(T):
                pst = psw.tile([128, 128], FP32, tag="psw", name="pst")
                nc.tensor.transpose(pst, wa[:, :, t], ident)
                nc.scalar.activation(
                    out=wT3[:, t, :],
                    in_=pst,
                    func=mybir.ActivationFunctionType.Copy,
                    scale=ymp1,
                )

            # demod scale: div[o] = rsqrt(sum_i s[i]*q[o,i] + eps),
            #   q[o,i] = sum_t w[o,i,t]^2,  s[i] = (1+y[i])^2
            wsq = const.tile([COUT, CIN, T], FP32, tag="wsq")
            nc.vector.tensor_mul(wsq, wa, wa)
            q = const.tile([COUT, CIN], FP32, tag="q")
            nc.vector.reduce_sum(q, wsq, axis=mybir.AxisListType.X)

            yp1row = const.tile([1, CIN], FP32, tag="yp1row")
            nc.vector.tensor_scalar_add(yp1row, yrow, 1.0)
            srow = const.tile([1, CIN], FP32, tag="srow")
            nc.vector.tensor_mul(srow, yp1row, yp1row)
            ones1 = const.tile([1, 128], FP32, tag="ones1")
            nc.vector.memset(ones1, 1.0)
            # broadcast srow across partitions via a K=1 matmul
            ps_s = psw.tile([128, 128], FP32, tag="psw", name="ps_s")
            nc.tensor.matmul(ps_s, ones1, srow, start=True, stop=True)

            tq = const.tile([COUT, CIN], FP32, tag="tq")
            nc.vector.tensor_mul(tq, q, ps_s)
            ncol = const.tile([COUT, 1], FP32, tag="ncol")
            nc.vector.reduce_sum(ncol, tq, axis=mybir.AxisListType.X)
            epst = const.tile([COUT, 1], FP32, tag="epst")
            nc.vector.memset(epst, EPS)
            sqn = const.tile([COUT, 1], FP32, tag="sqn")
            nc.scalar.activation(
                out=sqn,
                in_=ncol,
                func=mybir.ActivationFunctionType.Sqrt,
                bias=epst,
                scale=1.0,
            )
            div = const.tile([COUT, 1], FP32, tag="div")
            nc.vector.reciprocal(div, sqn)

            # ---------------- conv main loop ----------------
            stage_plane(0)
            stage_plane(1)
            for d in range(D):
                if d + 1 < D and pad_tiles[d + 1] is None:
                    stage_plane(d + 1)
                # valid kernel-depth offsets for this output plane
                mms = []
                for kd in range(K):
                    dp = d + kd - 1
                    if dp < 0 or dp >= D:
                        continue
                    for kh in range(K):
                        for kw in range(K):
                            mms.append((kd * 9 + kh * 3 + kw, dp, kh, kw))
                for hp in range(2):
                    r0 = hp * 16
                    ps = psum.tile([COUT, 512], FP32, tag="psum", name="ps")
                    for idx, (t, dp, kh, kw) in enumerate(mms):
                        rhs = pad_tiles[dp][:, r0 + kh : r0 + kh + 16, kw : kw + W]
                        nc.tensor.matmul(
                            ps,
                            wT3[:, t, :],
                            rhs,
                            start=(idx == 0),
                            stop=(idx == len(mms) - 1),
                        )
                    ot = evac.tile([COUT, 512], FP32, tag="evac", name="ot")
                    nc.scalar.activation(
                        out=ot,
                        in_=ps,
                        func=mybir.ActivationFunctionType.Copy,
                        scale=div,
                    )
                    nc.sync.dma_start(out=ov[:, d, r0 * W : r0 * W + 512], in_=ot)

    nc.compile()
    return nc


def _make_runner(nc):
    """Build the jitted 8-core executor once (mirrors
    bass2jax.run_bass_via_pjrt's multi-core path, but cacheable)."""
    import jax
    from jax.experimental.shard_map import shard_map
    from jax.sharding import Mesh, PartitionSpec

    from concourse import bass2jax

    bass2jax.install_neuronx_cc_hook()

    partition_name = (
        nc.partition_id_tensor.name if nc.partition_id_tensor else None
    )
    in_names, out_names, out_avals, zero_shapes = [], [], [], []
    for alloc in nc.m.functions[0].allocations:
        if not isinstance(alloc, mybir.MemoryLocationSet):
            continue
        name = alloc.memorylocations[0].name
        if alloc.kind == "ExternalInput":
            if name != partition_name:
                in_names.append(name)
        elif alloc.kind == "ExternalOutput":
            out_names.append(name)
            shape = tuple(alloc.tensor_shape)
            dtype = mybir.dt.np(alloc.dtype)
            out_avals.append(jax.core.ShapedArray(shape, dtype))
            zero_shapes.append((shape, dtype))
    n_params = len(in_names)
    n_outs = len(out_names)
    bind_in_names = in_names + out_names
    if partition_name is not None:
        bind_in_names = bind_in_names + [partition_name]
    bind_in_names = tuple(bind_in_names)
    donate = tuple(range(n_params, n_params + n_outs))

    def _body(*args):
        operands = list(args)
        if partition_name is not None:
            operands.append(bass2jax.partition_id_tensor())
        outs = bass2jax._bass_exec_p.bind(
            *operands,
            out_avals=tuple(out_avals),
            in_names=bind_in_names,
            out_names=tuple(out_names),
            lowering_input_output_aliases=(),
            sim_require_finite=True,
            sim_require_nnan=True,
            nc=nc,
        )
        return tuple(outs)

    devices = jax.devices()[:N_CORES]
    mesh = Mesh(np.asarray(devices), ("core",))
    in_specs = (PartitionSpec("core"),) * (n_params + n_outs)
    out_specs = (PartitionSpec("core"),) * n_outs
    sharded = jax.jit(
        shard_map(
            _body, mesh=mesh, in_specs=in_specs, out_specs=out_specs, check_rep=False
        ),
        donate_argnums=donate,
        keep_unused=True,
    )

    def run(in_maps):
        concat_in = [
            np.concatenate([np.asarray(m[n]) for m in in_maps], axis=0)
            for n in in_names
        ]
        concat_zeros = [
            np.zeros((N_CORES * s[0], *s[1:]), dt) for s, dt in zero_shapes
        ]
        out_arrs = sharded(*concat_in, *concat_zeros)
        return [
            {
                n: np.asarray(out_arrs[i]).reshape(N_CORES, *out_avals[i].shape)[c]
                for i, n in enumerate(out_names)
            }
            for c in range(N_CORES)
        ]

    return run


def kernel(x: np.ndarray, y: np.ndarray, weight: np.ndarray) -> np.ndarray:
    x = np.ascontiguousarray(np.asarray(x, dtype=np.float32))
    y = np.ascontiguousarray(np.asarray(y, dtype=np.float32))
    weight = np.ascontiguousarray(np.asarray(weight, dtype=np.float32))

    if "run" not in _CACHE:
        _CACHE["nc"] = _build_program()
        _CACHE["run"] = _make_runner(_CACHE["nc"])
    run = _CACHE["run"]

    wflat = weight.reshape(COUT, CIN, T)
    in_maps = [
        {"x": x[b], "y": y[b].reshape(CIN, 1), "w": wflat} for b in range(B)
    ]
    results = run(in_maps)
    out = np.stack(
        [results[b]["out"].reshape(COUT, D, H, W) for b in range(B)], axis=0
    )
    return out


if __name__ == "__main__":
    rng = np.random.default_rng(0)
    x = rng.standard_normal((B, CIN, D, H, W), dtype=np.float32)
    y = rng.standard_normal((B, CIN), dtype=np.float32)
    w = rng.standard_normal((COUT, CIN, K, K, K), dtype=np.float32) * 0.017
    out = kernel(x=x, y=y, weight=w)
    print("out", out.shape, out.dtype, float(np.abs(out).max()))



# revision 11
# speedup vs baseline: 1.4285x; 1.4285x over previous
"""Trainium2 Bass kernel for per-sample modulated+demodulated 3D conv.

Problem: x[B=8, CIN=128, 32,32,32], y[8,128], weight[128,128,3,3,3] (shared).
  w_b = weight * (1 + y[b,i]);  w_b *= rsqrt(sum_{i,k} w_b^2 + eps)  (per out-ch)
  out[b] = conv3d(x[b], w_b, same padding)

Sharding: data-parallel over batch, one sample per NeuronCore (8 cores).

Per core the conv is computed with 1D Winograd F(4,3) along W: the 3 W-taps
of each (kd,kh) become 6 winograd points over 8 output tiles of width 4, so
the tensor-engine work drops from 27 to 18 effective tap-matmuls per plane
(864 total matmuls of 512 free vs 1692 for direct conv).  The input
transform B^T (12 fused DVE ops per 2-plane chunk) and output transform A^T
(10 DVE ops per plane-pair) run on the vector engine, overlapped with the
matmuls; PSUM evacuation goes through the scalar engine.  Modulation is a
per-partition scale on the weight-transpose evacuation; demodulation is
folded into the winograd weights U.  Matmul operands are fp16 (fp32 PSUM).
"""

import sys

import numpy as np

try:
    import concourse.bass as bass
except ImportError:  # fresh grading dir: fall back to the repo checkout
    sys.path.insert(0, "/opt/trn_rl_repo")
    import concourse.bass as bass

import concourse.tile as tile
from concourse import bacc, mybir
from concourse.tile_rust import add_dep_helper
from concourse.masks import make_identity

B, CIN, COUT, K = 8, 128, 128, 3
D = H = W = 32
T = K * K * K  # 27
HP = H + 2  # x padded along H (host-side)
WP = W + 2  # x padded along W (host-side)
NP = 6  # winograd points F(4,3)
NT = 8  # winograd tiles along W (32/4)
VP = D + 2  # V planes incl zero D-padding
VH = H + 2  # V rows incl zero H-padding
EPS = 1e-8
N_CORES = 8

FP32 = mybir.dt.float32
FP16 = mybir.dt.float16
MM_DT = FP16

ALU = mybir.AluOpType

_CACHE = {}


def _build_program():
    nc = bacc.Bacc()
    xv = nc.dram_tensor("x", [CIN, D, HP, WP], FP16, kind="ExternalInput")
    yv = nc.dram_tensor("y", [CIN, 1], FP32, kind="ExternalInput")
    wv = nc.dram_tensor("w", [COUT, CIN, T], FP32, kind="ExternalInput")
    ov = nc.dram_tensor("out", [COUT, D, H * W], FP16, kind="ExternalOutput")

    with tile.TileContext(nc) as tc:
        with (
            tc.tile_pool(name="const", bufs=1) as const,
            tc.tile_pool(name="xc", bufs=2) as xcp,
            tc.tile_pool(name="vtmp", bufs=1) as vtmp,
            tc.tile_pool(name="wtap", bufs=2) as wtapp,
            tc.tile_pool(name="utmp", bufs=1) as utmp,
            tc.tile_pool(name="mst", bufs=2) as mst,
            tc.tile_pool(name="yo", bufs=2) as yop,
            tc.tile_pool(name="ot", bufs=1) as otp,
            tc.tile_pool(name="psum", bufs=6, space="PSUM") as psum,
            tc.tile_pool(name="psw", bufs=2, space="PSUM") as psw,
        ):
            # ---------------- weight + x DMAs up front ----------------
            wa = const.tile([COUT, CIN, T], FP32, tag="wa")
            nc.sync.dma_start(out=wa, in_=wv[:, :, :])
            ym = const.tile([CIN, 1], FP32, tag="ym")
            nc.sync.dma_start(out=ym, in_=yv[:, :])
            yrow = const.tile([1, CIN], FP32, tag="yrow")
            nc.sync.dma_start(out=yrow, in_=yv[:, 0])

            # V: winograd-domain input, zero borders for D/H same-padding.
            # [c, plane(0..33), p, row(0..33), t]
            V = const.tile([CIN, VP, NP, VH, NT], FP16, tag="V")

            xc_tiles = [None] * (D // 2)
            xc_dmas = [None] * (D // 2)
            tf_insts = [None] * (D // 2)

            def stage_chunk(c):
                xct = xcp.tile([CIN, 2, HP, WP], FP16, tag="xc", name="xct")
                dma = nc.sync.dma_start(out=xct, in_=xv[:, 2 * c : 2 * c + 2, :, :])
                xc_tiles[c] = xct
                xc_dmas[c] = dma

            stage_chunk(0)
            stage_chunk(1)

            # (1 + y[i]) per-partition column (modulation scale)
            ymp1 = const.tile([CIN, 1], FP32, tag="ymp1")
            nc.vector.tensor_scalar_add(ymp1, ym, 1.0)

            ident = const.tile([128, 128], FP32, tag="ident")
            make_identity(nc, ident)

            # ---------------- demod scale ----------------
            # div[o] = rsqrt(sum_i s[i]*q[o,i] + eps), q=sum_t w^2, s=(1+y)^2
            # (q computed in 3 chunks of 9 taps to bound scratch SBUF)
            wsq = const.tile([COUT, CIN, 9], FP32, tag="wsq")
            qp = const.tile([COUT, CIN], FP32, tag="qp")
            q = const.tile([COUT, CIN], FP32, tag="q")
            for g in range(3):
                sl = wa[:, :, 9 * g : 9 * g + 9]
                nc.vector.tensor_mul(wsq, sl, sl)
                if g == 0:
                    nc.vector.reduce_sum(q, wsq, axis=mybir.AxisListType.X)
                else:
                    nc.vector.reduce_sum(qp, wsq, axis=mybir.AxisListType.X)
                    nc.vector.tensor_add(q, q, qp)

            yp1row = const.tile([1, CIN], FP32, tag="yp1row")
            nc.vector.tensor_scalar_add(yp1row, yrow, 1.0)
            srow = const.tile([1, CIN], FP32, tag="srow")
            nc.vector.tensor_mul(srow, yp1row, yp1row)
            ones1 = const.tile([1, 128], FP32, tag="ones1")
            nc.vector.memset(ones1, 1.0)
            ps_s = psw.tile([128, 128], FP32, tag="psw", name="ps_s")
            nc.tensor.matmul(ps_s, ones1, srow, start=True, stop=True)

            tq = const.tile([COUT, CIN], FP32, tag="tq")
            nc.vector.tensor_mul(tq, q, ps_s)
            ncol = const.tile([COUT, 1], FP32, tag="ncol")
            nc.vector.reduce_sum(ncol, tq, axis=mybir.AxisListType.X)
            epst = const.tile([COUT, 1], FP32, tag="epst")
            nc.vector.memset(epst, EPS)
            sqn = const.tile([COUT, 1], FP32, tag="sqn")
            nc.scalar.activation(
                out=sqn,
                in_=ncol,
                func=mybir.ActivationFunctionType.Sqrt,
                bias=epst,
                scale=1.0,
            )
            div = const.tile([COUT, 1], FP32, tag="div")
            nc.vector.reciprocal(div, sqn)

            # ---------------- winograd weights U ----------------
            # U[c, kd*3+kh, p, o] = sum_kw G[p,kw] w[o,c,kd,kh,kw]*(1+y[c])
            # (demodulation div[o] is applied later, as the per-partition
            # scale on the PSUM evacuation copies)
            U = const.tile([CIN, 9, NP, COUT], FP16, tag="U")
            u_insts = []
            for kd in range(K):
                for kh in range(K):
                    i9 = kd * 3 + kh
                    wt = wtapp.tile([CIN, 3, COUT], FP32, tag="wt", name="wt")
                    wtacts = []
                    for kw in range(K):
                        t27 = kd * 9 + kh * 3 + kw
                        pst = psw.tile([128, 128], FP32, tag="psw", name="pst")
                        nc.tensor.transpose(pst, wa[:, :, t27], ident)
                        wtacts.append(nc.scalar.activation(
                            out=wt[:, kw, :],
                            in_=pst,
                            func=mybir.ActivationFunctionType.Copy,
                            scale=ymp1,
                        ))
                    w0, w1, w2 = wt[:, 0, :], wt[:, 1, :], wt[:, 2, :]
                    s_ = utmp.tile([CIN, 128], FP32, tag="s_", name="s_")
                    t1 = utmp.tile([CIN, 128], FP32, tag="t1", name="t1")
                    t2 = utmp.tile([CIN, 128], FP32, tag="t2", name="t2")
                    a2 = utmp.tile([CIN, 128], FP32, tag="a2", name="a2")
                    b2 = utmp.tile([CIN, 128], FP32, tag="b2", name="b2")
                    # u0 = 1/4 w0
                    u_first = nc.vector.tensor_scalar_mul(U[:, i9, 0, :], w0, 0.25)
                    for a in wtacts:
                        add_dep_helper(u_first.ins, a.ins, reason="wt act -> U chain")
                    u_insts.append(u_first)
                    # u1 = -1/6 (w0+w1+w2) ; u2 = -1/6 (w0-w1+w2)
                    nc.vector.tensor_add(s_, w0, w2)
                    nc.vector.tensor_add(t1, s_, w1)
                    u_insts.append(nc.vector.tensor_scalar_mul(U[:, i9, 1, :], t1, -1.0 / 6.0))
                    nc.vector.tensor_sub(t2, s_, w1)
                    u_insts.append(nc.vector.tensor_scalar_mul(U[:, i9, 2, :], t2, -1.0 / 6.0))
                    # u3 = 1/24 (w0 + 2 w1 + 4 w2) ; u4 = 1/24 (w0 - 2 w1 + 4 w2)
                    nc.vector.scalar_tensor_tensor(
                        a2, w1, 2.0, w0, op0=ALU.mult, op1=ALU.add
                    )
                    nc.vector.scalar_tensor_tensor(
                        a2, w2, 4.0, a2, op0=ALU.mult, op1=ALU.add
                    )
                    u_insts.append(nc.vector.tensor_scalar_mul(U[:, i9, 3, :], a2, 1.0 / 24.0))
                    nc.vector.scalar_tensor_tensor(
                        b2, w1, -2.0, w0, op0=ALU.mult, op1=ALU.add
                    )
                    nc.vector.scalar_tensor_tensor(
                        b2, w2, 4.0, b2, op0=ALU.mult, op1=ALU.add
                    )
                    u_insts.append(nc.vector.tensor_scalar_mul(U[:, i9, 4, :], b2, 1.0 / 24.0))
                    # u5 = w2
                    u_insts.append(nc.vector.tensor_copy(U[:, i9, 5, :], w2))

            # ---------------- V border zeros ----------------
            # D-padding planes 0 and 33; H-padding rows 0 and 33 of planes 1..32
            vz0 = nc.gpsimd.memset(V[:, 0, :, :, :], 0.0)
            vz1 = nc.gpsimd.memset(V[:, VP - 1, :, :, :], 0.0)

            # ---------------- input transform (2 planes per chunk) -------
            # d_s = x[:, :, :, s::4] (8 tiles); BT factored, all fp16
            def transform_chunk(c):
                if xc_tiles[c] is None:
                    stage_chunk(c)
                xct = xc_tiles[c]
                ds = [xct[:, :, :, s : s + 29 : 4] for s in range(6)]
                vout = V[:, 2 * c + 1 : 2 * c + 3, :, :, :]

                def vt(tag):
                    return vtmp.tile([CIN, 2, VH, NT], FP16, tag=tag, name=tag)

                cc, ee, tt1, aa, bb, tt2 = (
                    vt("cc"), vt("ee"), vt("tt1"), vt("aa"), vt("bb"), vt("tt2"),
                )
                stt = nc.vector.scalar_tensor_tensor
                insts = []
                # v0 = 4 d0 - 5 d2 + d4
                insts.append(stt(tt1, ds[2], -5.0, ds[4], op0=ALU.mult, op1=ALU.add))
                insts.append(stt(vout[:, :, 0, :, :], ds[0], 4.0, tt1, op0=ALU.mult, op1=ALU.add))
                # v1 = (d3 - 4 d1) + (d4 - 4 d2); v2 = (d4 - 4 d2) - (d3 - 4 d1)
                insts.append(stt(aa, ds[1], -4.0, ds[3], op0=ALU.mult, op1=ALU.add))
                insts.append(stt(bb, ds[2], -4.0, ds[4], op0=ALU.mult, op1=ALU.add))
                insts.append(nc.vector.tensor_add(vout[:, :, 1, :, :], aa, bb))
                insts.append(nc.vector.tensor_sub(vout[:, :, 2, :, :], bb, aa))
                # v3 = 2(d3-d1) + (d4-d2); v4 = -2(d3-d1) + (d4-d2)
                insts.append(nc.vector.tensor_sub(cc, ds[3], ds[1]))
                insts.append(nc.vector.tensor_sub(ee, ds[4], ds[2]))
                insts.append(stt(vout[:, :, 3, :, :], cc, 2.0, ee, op0=ALU.mult, op1=ALU.add))
                insts.append(stt(vout[:, :, 4, :, :], cc, -2.0, ee, op0=ALU.mult, op1=ALU.add))
                # v5 = 4 d1 - 5 d3 + d5
                insts.append(stt(tt2, ds[3], -5.0, ds[5], op0=ALU.mult, op1=ALU.add))
                insts.append(stt(vout[:, :, 5, :, :], ds[1], 4.0, tt2, op0=ALU.mult, op1=ALU.add))
                # explicit cross-engine RAW: transform reads this chunk's DMA
                add_dep_helper(insts[0].ins, xc_dmas[c].ins, reason="xc dma -> BT")
                tf_insts[c] = [i.ins for i in insts]

            transform_chunk(0)
            transform_chunk(1)

            # ---------------- main loop: 16 plane-pairs ----------------
            for pr in range(D // 2):
                d0 = 2 * pr
                # keep the input pipeline ~2 chunks ahead
                nx = pr + 2
                if nx < D // 2 and tf_insts[nx] is None:
                    if nx + 1 < D // 2 and xc_tiles[nx + 1] is None:
                        stage_chunk(nx + 1)
                    transform_chunk(nx)

                mt = mst.tile([COUT, NP, 2, H, NT], FP16, tag="mt", name="mt")
                mtacts = []
                first_mm = None
                for p in range(NP):
                    ps = psum.tile(
                        [COUT, 2, H, NT], FP32, tag="mm", name=f"ps{p}"
                    )
                    idx = 0
                    for kd in range(K):
                        rhs = V[:, d0 + kd : d0 + kd + 2, p, :, :]
                        for kh in range(K):
                            mm = nc.tensor.matmul(
                                ps,
                                U[:, kd * 3 + kh, p, :],
                                rhs[:, :, kh : kh + H, :],
                                start=(idx == 0),
                                stop=(idx == 8),
                            )
                            if first_mm is None:
                                first_mm = mm
                            idx += 1
                    mtacts.append(nc.scalar.activation(
                        out=mt[:, p, :, :, :],
                        in_=ps,
                        func=mybir.ActivationFunctionType.Copy,
                        scale=div,
                    ))
                # explicit cross-engine RAW edges the tracker misses:
                # pair pr reads V planes 2pr..2pr+3 (chunks pr-1, pr, pr+1
                # transforms; chunk pr-1 covered transitively via prev pair)
                for cdep in (pr, pr + 1):
                    if 0 <= cdep < D // 2 and tf_insts[cdep] is not None:
                        for wi in tf_insts[cdep]:
                            add_dep_helper(first_mm.ins, wi, reason="V BT -> mm")
                if pr == 0:
                    add_dep_helper(first_mm.ins, vz0.ins, reason="V zero -> mm")
                    for ui in u_insts:
                        add_dep_helper(first_mm.ins, ui.ins, reason="U -> mm")
                if pr == D // 2 - 1:
                    add_dep_helper(first_mm.ins, vz1.ins, reason="V zero -> mm")

                # output transform A^T (fp16)
                def ot(tag):
                    return otp.tile([COUT, 2, H, NT], FP16, tag=tag, name=tag)

                s, dd, t, u, q1, t3 = (
                    ot("s"), ot("dd"), ot("t"), ot("u"), ot("q1"), ot("t3"),
                )
                m = [mt[:, p, :, :, :] for p in range(NP)]
                yt = yop.tile([COUT, 2, H, W], FP16, tag="yt", name="yt")
                stt = nc.vector.scalar_tensor_tensor
                ot0 = nc.vector.tensor_add(s, m[1], m[2])
                for a in mtacts:
                    add_dep_helper(ot0.ins, a.ins, reason="mt act -> AT")
                nc.vector.tensor_sub(dd, m[1], m[2])
                nc.vector.tensor_add(t, m[3], m[4])
                nc.vector.tensor_sub(u, m[3], m[4])
                # y0 = m0 + s + t
                nc.vector.tensor_add(q1, s, t)
                yw = [nc.vector.tensor_add(yt[:, :, :, 0::4], q1, m[0])]
                # y1 = dd + 2u ; y2 = s + 4t ; y3 = dd + 8u + m5
                yw.append(stt(yt[:, :, :, 1::4], u, 2.0, dd, op0=ALU.mult, op1=ALU.add))
                yw.append(stt(yt[:, :, :, 2::4], t, 4.0, s, op0=ALU.mult, op1=ALU.add))
                stt(t3, u, 8.0, dd, op0=ALU.mult, op1=ALU.add)
                yw.append(nc.vector.tensor_add(yt[:, :, :, 3::4], t3, m[5]))

                odma = nc.sync.dma_start(
                    out=ov[:, d0 : d0 + 2, :], in_=yt
                )
                for w in yw:
                    add_dep_helper(odma.ins, w.ins, reason="AT -> out dma")

    nc.compile()
    return nc


def _make_runner(nc):
    """Build the jitted 8-core executor once (mirrors
    bass2jax.run_bass_via_pjrt's multi-core path, but cacheable)."""
    import jax
    from jax.experimental.shard_map import shard_map
    from jax.sharding import Mesh, PartitionSpec

    from concourse import bass2jax

    bass2jax.install_neuronx_cc_hook()

    partition_name = (
        nc.partition_id_tensor.name if nc.partition_id_tensor else None
    )
    in_names, out_names, out_avals, zero_shapes = [], [], [], []
    for alloc in nc.m.functions[0].allocations:
        if not isinstance(alloc, mybir.MemoryLocationSet):
            continue
        name = alloc.memorylocations[0].name
        if alloc.kind == "ExternalInput":
            if name != partition_name:
                in_names.append(name)
        elif alloc.kind == "ExternalOutput":
            out_names.append(name)
            shape = tuple(alloc.tensor_shape)
            dtype = mybir.dt.np(alloc.dtype)
            out_avals.append(jax.core.ShapedArray(shape, dtype))
            zero_shapes.append((shape, dtype))
    n_params = len(in_names)
    n_outs = len(out_names)
    bind_in_names = in_names + out_names
    if partition_name is not None:
        bind_in_names = bind_in_names + [partition_name]
    bind_in_names = tuple(bind_in_names)
    donate = tuple(range(n_params, n_params + n_outs))

    def _body(*args):
        operands = list(args)
        if partition_name is not None:
            operands.append(bass2jax.partition_id_tensor())
        outs = bass2jax._bass_exec_p.bind(
            *operands,
            out_avals=tuple(out_avals),
            in_names=bind_in_names,
            out_names=tuple(out_names),
            lowering_input_output_aliases=(),
            sim_require_finite=True,
            sim_require_nnan=True,
            nc=nc,
        )
        return tuple(outs)

    devices = jax.devices()[:N_CORES]
    mesh = Mesh(np.asarray(devices), ("core",))
    in_specs = (PartitionSpec("core"),) * (n_params + n_outs)
    out_specs = (PartitionSpec("core"),) * n_outs
    sharded = jax.jit(
        shard_map(
            _body, mesh=mesh, in_specs=in_specs, out_specs=out_specs, check_rep=False
        ),
        donate_argnums=donate,
        keep_unused=True,
    )

    def run(in_maps):
        concat_in = [
            np.concatenate([np.asarray(m[n]) for m in in_maps], axis=0)
            for n in in_names
        ]
        concat_zeros = [
            np.zeros((N_CORES * s[0], *s[1:]), dt) for s, dt in zero_shapes
        ]
        out_arrs = sharded(*concat_in, *concat_zeros)
        return [
            {
                n: np.asarray(out_arrs[i]).reshape(N_CORES, *out_avals[i].shape)[c]
                for i, n in enumerate(out_names)
            }
            for c in range(N_CORES)
        ]

    return run


def make_in_maps(x, y, weight):
    """Host-side input formatting: fp16 + zero-pad x along W."""
    x = np.asarray(x, dtype=np.float32)
    y = np.asarray(y, dtype=np.float32)
    weight = np.asarray(weight, dtype=np.float32)
    xpad = np.zeros((B, CIN, D, HP, WP), dtype=np.float16)
    xpad[:, :, :, 1 : H + 1, 1 : W + 1] = x.astype(np.float16)
    wflat = np.ascontiguousarray(weight.reshape(COUT, CIN, T))
    return [
        {"x": np.ascontiguousarray(xpad[b]), "y": np.ascontiguousarray(y[b].reshape(CIN, 1)), "w": wflat}
        for b in range(B)
    ]


def kernel(x: np.ndarray, y: np.ndarray, weight: np.ndarray) -> np.ndarray:
    if "run" not in _CACHE:
        _CACHE["nc"] = _build_program()
        _CACHE["run"] = _make_runner(_CACHE["nc"])
    run = _CACHE["run"]

    results = run(make_in_maps(x, y, weight))
    out = np.stack(
        [
            results[b]["out"].reshape(COUT, D, H, W).astype(np.float32)
            for b in range(B)
        ],
        axis=0,
    )
    return out


if __name__ == "__main__":
    rng = np.random.default_rng(0)
    x = rng.standard_normal((B, CIN, D, H, W), dtype=np.float32)
    y = rng.standard_normal((B, CIN), dtype=np.float32)
    w = rng.standard_normal((COUT, CIN, K, K, K), dtype=np.float32) * 0.017
    out = kernel(x=x, y=y, weight=w)
    print("out", out.shape, out.dtype, float(np.abs(out).max()))
